# revision 17
# baseline (speedup 1.0000x reference)
"""Trainium2 Bass kernel for the DINO-style CorrelationLoss.

Math:
  loss = dino + 5.0 * corr
  dino = (1/18) * sum_{(t,s): t_ix != s_ix} M[t,s]
  M[t,s] = -(1/B) sum_b [ dot(t_p[t,b], x_s[s,b]) / Ts - LSE(x_s[s,b]/Ts) ]
with t_p = softmax((teacher-center)/Tt) over d, LSE = log-sum-exp.

Key identities exploited on-chip:
  - t_p[t] = e_t / Z_t with e_t = exp(25*(te-c) - 88), Z_t = sum_d e_t
    (the fixed shift cancels in the ratio; center is subtracted on the
    host so e_t comes from ONE activation pass).
  - The allowed-pair sum collapses: s>=2 pairs with both teachers, so
      sum_t dot(t_p[t], x_s) = dot(w, x_s),  w = e_0/Z_0 + e_1/Z_1,
    computed once on-chip (PE broadcasts Z across the 8 d-octant
    partitions with a tiny block-diagonal ones matmul; DVE reciprocal
    + two 4x tensor_scalar muls build w).  s=0 pairs only with t=1 and
    s=1 only with t=0 -> two raw e_t dots, normalized by Z on host.
  => 10 big elementwise muls instead of 18 (and none for the skipped
     diagonal pairs the reference masks out).

Per-core layout (batch sharded 8 ways, 16 samples/core):
  partition p = b*8 + c (c = d-octant), free = 8192 d-elems per octant.
  Both inputs marshalled to bf16 on host -> 25.2 MB HBM traffic/core.

Engine budget per core (v2 trace-calibrated):
  ACT  12 full-row-equivalent exps @7.1us + accum reads ~ 87 us <- pacer
  DVE  10 dot muls @4.42 + w-build + psum evicts       ~ 60 us
  PE   10x16 block-ones reduce matmuls + heartbeats    ~ 40 us
  DMA  25.2 MB                                         ~ 70-105 us
LSE/Z reductions ride ACT's accum_out (free); dot reductions go
through PE block-ones matmuls into psum strips (the DVE fused
reduce opcode measured 1x on HW - avoid). Heartbeat matmuls pinned to
teacher/w dependencies keep the PE HAM clock from re-throttling during
the startup phase. Host finishes the tiny cross-octant/residual sums
and final algebra in float64, plus the 10x10 crop-0 correlation block
from the original f32 input.
"""

import numpy as np
import ml_dtypes

import concourse.bass as bass
import concourse.bacc as bacc
import concourse.tile as tile
from concourse import mybir
from concourse.bass_utils import run_bass_kernel_spmd

# problem constants (hardcoded; kernel.py must be self-contained)
NS, NT, B, D = 10, 2, 128, 65536
NCORES = 8
BL = B // NCORES            # 16 samples per core
C8 = 8                      # d-octants per sample -> partition packing
FTOT = D // C8              # 8192 free elems per partition
TCH = 2048                  # teacher chunk free elems (4 chunks per crop)
NTCH = FTOT // TCH          # 4
NSLICE = FTOT // 512        # 16 psum-width slices per dot group
NGRP = NS                   # 10 dot groups (2 raw + 8 combined)
NWARM = 24
STUDENT_TEMP = 0.1
TEACHER_TEMP = 0.04
MARGIN = 0.7
CORR_WEIGHT = 5.0
TSHIFT = 88.0

# out_partials column map
COL_LSE = 0                   # cols 0..9   : sum_f exp(10*x_s) per crop
COL_ZP = 10                   # cols 10..17 : Z partials (t,chunk)
COL_NZ = NT * NTCH            # 8
NCOL = COL_ZP + COL_NZ        # 18

F32 = mybir.dt.float32
BF16 = mybir.dt.bfloat16

_CACHED = None


def _build_module():
    nc = bacc.Bacc("TRN2", target_bir_lowering=False, debug=False)
    student = nc.declare_dram_parameter("student", [NS, BL, D], mybir.dt.float8e4, isOutput=False)
    # teacher is center-subtracted on the host
    teacher = nc.declare_dram_parameter("teacher", [NT, BL, D], BF16, isOutput=False)
    # block-diagonal ones (16 groups of 8): bcast[m] = sum_{p in group(m)} z[p]
    sdiag = nc.declare_dram_parameter("sdiag", [128, 128], F32, isOutput=False)
    blockones = nc.declare_dram_parameter("blockones", [128, 16], BF16, isOutput=False)
    out_dots = nc.declare_dram_parameter("out_dots", [NGRP, 16, 512], F32, isOutput=True)
    out_partials = nc.declare_dram_parameter("out_partials", [128, NCOL], F32, isOutput=True)

    xviews = [student[s].rearrange("b (c f) -> (b c) f", c=C8) for s in range(NS)]
    tview = teacher.rearrange("t b (c f) -> (b c) t f", c=C8)

    from contextlib import ExitStack

    MULT = mybir.AluOpType.mult

    with tile.TileContext(nc) as tc:
        with ExitStack() as stack:
            consts = stack.enter_context(tc.tile_pool(name="consts", bufs=1))
            pers = stack.enter_context(tc.tile_pool(name="pers", bufs=1))
            xb_pool = stack.enter_context(tc.tile_pool(name="xb", bufs=4))
            ev_pool = stack.enter_context(tc.tile_pool(name="ev", bufs=2))
            psum_pool = stack.enter_context(
                tc.tile_pool(name="psum", bufs=1, space=bass.MemorySpace.PSUM)
            )

            bo = consts.tile([128, 16], BF16, tag="bo")
            junk = consts.tile([128, 512], BF16, tag="junk")
            nc.gpsimd.memset(junk[:], 0.0)
            junk16 = consts.tile([128, 16], BF16, tag="junk16")
            nc.gpsimd.memset(junk16[:], 0.0)
            bias0 = consts.tile([128, 1], F32, tag="bias0")
            nc.gpsimd.memset(bias0[:], 0.0)
            biasK = consts.tile([128, 1], F32, tag="biasK")
            nc.gpsimd.memset(biasK[:], -TSHIFT)
            sd = consts.tile([128, 128], F32, tag="sdiag")
            partials = consts.tile([128, NCOL], F32, tag="partials")
            zsum = consts.tile([128, 2], F32, tag="zsum")
            rbc = consts.tile([128, 2], F32, tag="rbc")

            # preload the Exp activation table before the DMA stream
            # saturates the fabric (implicit table load rides this dummy)
            tabw = consts.tile([128, 8], BF16, tag="tabw")
            nc.scalar.activation(
                tabw[:], junk16[:, 0:8], mybir.ActivationFunctionType.Exp,
                bias=bias0[:], scale=1.0,
            )

            accJ = psum_pool.tile([128, 512], F32, tag="accJ", name="accJ")
            accs = [
                psum_pool.tile([128, 512], F32, tag=f"acc{i}", name=f"acc{i}")
                for i in range(3)
            ]
            zbc = psum_pool.tile([128, 2], F32, tag="zbc")

            et = [
                pers.tile([128, FTOT], BF16, tag=f"et{t}", name=f"et{t}")
                for t in range(NT)
            ]
            w = pers.tile([128, FTOT], BF16, tag="w")

            def heartbeat(src, stat=None):
                # junk matmul whose moving operand depends on real work --
                # keeps the PE HAM clock alive without free-running
                nc.tensor.matmul(
                    accJ[0:16, :], (stat if stat is not None else bo)[:], src,
                    start=True, stop=True, skip_group_check=True,
                    tile_position=(0, 0),
                )

            # PE warm-up while DMAs stream in (memset stationary so no DMA
            # sits ahead of the teacher stream)
            for _ in range(NWARM):
                heartbeat(junk[:], stat=junk16)

            # startup-only pool: freed before the crop-loop pools open
            start_scope = ExitStack()
            tch_pool = start_scope.enter_context(tc.tile_pool(name="tch", bufs=8))
            wtmp_pool = start_scope.enter_context(tc.tile_pool(name="wtmp", bufs=2))

            # ---- teacher: 2x4 chunks so ACT starts early; Z partials ride
            # the exp's accumulator ----
            for t in range(NT):
                for h in range(NTCH):
                    tr = tch_pool.tile([128, TCH], BF16, name="tch")
                    nc.gpsimd.dma_start(tr[:], tview[:, t, h * TCH:(h + 1) * TCH])
                    nc.scalar.activation(
                        et[t][:, h * TCH:(h + 1) * TCH], tr[:],
                        mybir.ActivationFunctionType.Exp,
                        bias=biasK[:], scale=1.0 / TEACHER_TEMP,
                        accum_out=partials[:, COL_ZP + t * NTCH + h:COL_ZP + t * NTCH + h + 1],
                    )
                    heartbeat(et[t][:, h * TCH:h * TCH + 512], stat=junk16)
            nc.sync.dma_start(bo[:], blockones[:])
            nc.sync.dma_start(sd[:], sdiag[:])

            # ---- Z -> 1/Z broadcast across each sample's 8 partitions ----
            for t in range(NT):
                nc.vector.tensor_reduce(
                    zsum[:, t:t + 1],
                    partials[:, COL_ZP + t * NTCH:COL_ZP + (t + 1) * NTCH],
                    mybir.AxisListType.XYZW,
                    mybir.AluOpType.add,
                )
            nc.tensor.matmul(
                zbc[:, 0:2], sd[:], zsum[:, 0:2],
                start=True, stop=True, skip_group_check=True,
                tile_position=(0, 0),
            )
            nc.vector.reciprocal(rbc[:], zbc[:, 0:2])

            # ---- w = e0/Z0 + e1/Z1 (per-partition scalars, 4x DVE) ----
            w0 = wtmp_pool.tile([128, FTOT], BF16, name="wtmp")
            nc.vector.tensor_scalar(w0[:], et[0][:], rbc[:, 0:1], None, MULT)
            heartbeat(w0[:, 0:512])
            w1 = wtmp_pool.tile([128, FTOT], BF16, name="wtmp")
            nc.vector.tensor_scalar(w1[:], et[1][:], rbc[:, 1:2], None, MULT)
            nc.vector.tensor_add(w[:], w0[:], w1[:])
            heartbeat(w[:, 0:512])
            start_scope.close()

            expx_pool = stack.enter_context(tc.tile_pool(name="expx", bufs=2))
            pp_pool = stack.enter_context(tc.tile_pool(name="pp", bufs=2))

            # ---- student crops (single queue, strict priority order: the
            # DMA fabric is chip-shared ~280 GB/s; a second queue only
            # breaks ordering) ----
            for s in range(NS):
                xb = xb_pool.tile([128, FTOT], BF16, name="xb")
                nc.gpsimd.dma_start(xb[:], xviews[s][:])
                expx = expx_pool.tile([128, FTOT], BF16, name="expx")
                nc.scalar.activation(
                    expx[:], xb[:], mybir.ActivationFunctionType.Exp,
                    bias=bias0[:], scale=1.0 / STUDENT_TEMP,
                    accum_out=partials[:, COL_LSE + s:COL_LSE + s + 1],
                )
                bsrc = et[1] if s == 0 else (et[0] if s == 1 else w)
                pp = pp_pool.tile([128, FTOT], BF16, name="pp")
                nc.vector.tensor_mul(pp[:], bsrc[:], xb[:])
                bank, strip = accs[s // 4], 32 * (s % 4)
                for sl in range(NSLICE):
                    nc.tensor.matmul(
                        bank[strip:strip + 16, :],
                        bo[:],
                        pp[:, sl * 512:(sl + 1) * 512],
                        start=(sl == 0),
                        stop=(sl == NSLICE - 1),
                        skip_group_check=True,
                        tile_position=(0, strip),
                    )
                ev = ev_pool.tile([16, 512], F32, name="ev")
                nc.vector.tensor_copy(ev[:], bank[strip:strip + 16, :])
                nc.sync.dma_start(out_dots[s], ev[:])

            nc.sync.dma_start(out_partials[:], partials[:])

    nc.compile()
    return nc


def _get_module():
    global _CACHED
    if _CACHED is None:
        _CACHED = _build_module()
    return _CACHED


def _sdiag_np():
    sd = np.zeros((128, 128), dtype=np.float32)
    for p in range(128):
        g = p // C8
        sd[p, g * C8:(g + 1) * C8] = 1.0
    return sd


def _blockones_np():
    bo = np.zeros((128, 16), dtype=ml_dtypes.bfloat16)
    for p in range(128):
        bo[p, p // C8] = 1.0
    return bo


def _prep_in_maps(student_output, teacher_output, center):
    student_bf = np.asarray(student_output, dtype=np.float32).astype(ml_dtypes.float8_e4m3)
    tc = np.asarray(teacher_output, dtype=np.float32) - np.asarray(center, dtype=np.float32)[None]
    teacher_bf = tc.astype(ml_dtypes.bfloat16)
    sd = _sdiag_np()
    bo = _blockones_np()
    in_maps = []
    for core in range(NCORES):
        b0 = core * BL
        in_maps.append({
            "student": np.ascontiguousarray(student_bf[:, b0:b0 + BL, :]),
            "teacher": np.ascontiguousarray(teacher_bf[:, b0:b0 + BL, :]),
            "sdiag": sd,
            "blockones": bo,
        })
    return in_maps


def kernel(student_output, teacher_output, center):
    nc = _get_module()
    in_maps = _prep_in_maps(student_output, teacher_output, center)
    res = run_bass_kernel_spmd(nc, in_maps, list(range(NCORES))).results

    # ---- host combine (tiny reductions + final algebra, float64) ----
    terms = 0.0  # sum_b of (combined dot term)
    lses = 0.0   # sum_b of weighted LSE sum
    for core in range(NCORES):
        pc = (
            np.asarray(res[core]["out_partials"], dtype=np.float64)
            .reshape(BL, C8, NCOL)
            .sum(axis=1)
        )  # [16, NCOL]
        lse = np.log(pc[:, COL_LSE:COL_LSE + NS])            # [16, 10]
        z = pc[:, COL_ZP:COL_ZP + COL_NZ].reshape(BL, NT, NTCH).sum(axis=2)  # [16, 2]
        dots = np.asarray(res[core]["out_dots"], dtype=np.float64).sum(axis=2)  # [10, 16]
        d10 = dots[0]                                        # dot(e1, x0)
        d01 = dots[1]                                        # dot(e0, x1)
        wd = dots[2:]                                        # [8, 16]
        terms += (d10 / z[:, 1] + d01 / z[:, 0] + wd.sum(axis=0)).sum()
        lses += (lse[:, 0] + lse[:, 1] + 2.0 * lse[:, 2:].sum(axis=1)).sum()

    n_dino_terms = NT * NS - min(NT, NS)
    dino = -(terms / STUDENT_TEMP - lses) / (n_dino_terms * B)

    e0 = np.asarray(student_output, dtype=np.float32)[0, :NS].astype(np.float64)
    e0 = e0 / np.maximum(np.linalg.norm(e0, axis=-1, keepdims=True), 1e-12)
    sim = e0 @ e0.T
    iu = np.triu(np.ones((NS, NS)), k=1)
    corr = (np.maximum(sim - (1.0 - MARGIN), 0.0) * iu).sum() / (NS * (NS - 1) // 2)

    return np.float32(dino + CORR_WEIGHT * corr)
